# revision 11
# baseline (speedup 1.0000x reference)
"""CapsNet dynamic-routing layer on 8 Trainium2 NeuronCores.

Sharding: tensor-parallel over num_caps_j (J=32 -> 4 per core). Every
(batch, j) pair's routing is independent, so there are no collectives:
each core computes u_hat[:, :, j_shard, :] plus 3 routing iterations and
returns v_J[:, j_shard, :].

Phase 1 (einsum, DMA-bound ~67MB/core): 1024 PE matmuls
(stationary = x chunk [d,b], moving = W [d,jv], N=128) accumulate
u_hat into PSUM; the scalar engine copies each PSUM block to SBUF in
TRANSPOSED layout U[b, j, v, i] (i innermost). s0 = sum_i u_hat is
folded per block on the otherwise-idle DVE.

Phase 2 (routing, 2 iterations). The [b, j, v, i] layout gives:
  - t-pass product w_v*U_v on the SCALAR engine (scale is the
    per-partition [B,1] AP it natively supports), v-fold on DVE at
    2x (v is an outer axis -> contiguous halves).
  - exp(b_r) is only [B, JL, I] (2M elems vs 8.4M broadcast before).
  - s-pass product U * exp(b) on DVE at 2x: exp(b)[B,I] broadcast
    over v is a stride-0 OUTER dim, which keeps the innermost stride
    at 1 (the 2x_1p requirement); i-fold halves stay contiguous.
"""

import sys

if "/opt/trn_rl_repo" not in sys.path:
    sys.path.insert(0, "/opt/trn_rl_repo")

import numpy as np

B, I, D, J, V = 128, 512, 256, 32, 32
NCORES = 8
JL = J // NCORES          # 4 j's per core
JV = JL * V               # 128
DP = 128                  # contraction chunk (partitions)
EPS = 1e-9
IBLK = 16                 # i-block per DMA tile / PSUM block
VQ = 8                    # v-quarter for t/s pass tiles

_cache = {}


def _build_program():
    import concourse.tile as tile
    from concourse import bacc, mybir

    f16 = mybir.dt.float16
    f32 = mybir.dt.float32
    MULT = mybir.AluOpType.mult

    nc = bacc.Bacc("TRN2", target_bir_lowering=False, debug=False,
                   num_devices=NCORES)

    xa = nc.dram_tensor("xa", [DP, I, B], f16, kind="ExternalInput")
    xb = nc.dram_tensor("xb", [DP, I, B], f16, kind="ExternalInput")
    wa = nc.dram_tensor("wa", [DP, I, JV], f16, kind="ExternalInput")
    wb = nc.dram_tensor("wb", [DP, I, JV], f16, kind="ExternalInput")
    v2d = nc.dram_tensor("v2", [B, JV], f32, kind="ExternalOutput")

    with tile.TileContext(nc) as tc:
        from contextlib import ExitStack
        stack = ExitStack()
        upool = stack.enter_context(tc.tile_pool(name="uhat", bufs=1))
        xwpool = stack.enter_context(tc.tile_pool(name="xw", bufs=2))
        pspool = stack.enter_context(
            tc.tile_pool(name="psum", bufs=2, space="PSUM"))
        rpool = stack.enter_context(tc.tile_pool(name="rout", bufs=1))
        tpool = stack.enter_context(tc.tile_pool(name="tp", bufs=2))
        fpool = stack.enter_context(tc.tile_pool(name="fold", bufs=1))

        eps_t = rpool.tile([B, 1], f32, tag="eps")
        nc.gpsimd.memset(eps_t[:], EPS)

        # u_hat, [b, j, v, i] fp16 (i innermost)
        U = upool.tile([B, JL, V, I], f16)
        s0 = rpool.tile([B, JV], f32, tag="s0")

        def fold_scratch(n):
            """Flat [B, n] fp16 scratch keyed by size."""
            return fpool.tile([B, n], f16, tag=f"f{n}", name=f"fs{n}")

        # ---- einsum with transposed PSUM->SBUF copy + inline s0 ----
        for blk in range(I // IBLK):
            i0 = blk * IBLK
            xa_t = xwpool.tile([DP, IBLK, B], f16, tag="xa")
            nc.sync.dma_start(xa_t[:], xa.ap()[:, i0:i0 + IBLK, :])
            xb_t = xwpool.tile([DP, IBLK, B], f16, tag="xb")
            nc.sync.dma_start(xb_t[:], xb.ap()[:, i0:i0 + IBLK, :])
            wa_t = xwpool.tile([DP, IBLK, JV], f16, tag="wa")
            nc.gpsimd.dma_start(wa_t[:], wa.ap()[:, i0:i0 + IBLK, :])
            wb_t = xwpool.tile([DP, IBLK, JV], f16, tag="wb")
            nc.gpsimd.dma_start(wb_t[:], wb.ap()[:, i0:i0 + IBLK, :])

            ps = pspool.tile([B, IBLK, JV], f32)   # four 2KB banks
            for k in range(IBLK):
                nc.tensor.matmul(
                    ps[:, k, :], xa_t[:, k, :], wa_t[:, k, :],
                    start=True, stop=False)
                nc.tensor.matmul(
                    ps[:, k, :], xb_t[:, k, :], wb_t[:, k, :],
                    start=False, stop=True)
            # transpose (i, j, v) -> (j, v, i) on the scalar engine
            nc.scalar.copy(
                U[:, :, :, i0:i0 + IBLK],
                ps.rearrange("p i (j v) -> p j v i", j=JL))

            # s0 partial: fold this i-block (innermost halves, 2x)
            ub = U[:, :, :, i0:i0 + IBLK]
            t1 = fold_scratch(JV * 8)
            t1v = t1[:].rearrange("p (jv i) -> p jv i", i=8)
            nc.vector.tensor_add(t1v, ub.rearrange("p j v i -> p (j v) i")
                                 [:, :, 0:8],
                                 ub.rearrange("p j v i -> p (j v) i")
                                 [:, :, 8:16])
            t2 = fold_scratch(JV * 4)
            t2v = t2[:].rearrange("p (jv i) -> p jv i", i=4)
            nc.vector.tensor_add(t2v, t1v[:, :, 0:4], t1v[:, :, 4:8])
            t3 = fold_scratch(JV * 2)
            t3v = t3[:].rearrange("p (jv i) -> p jv i", i=2)
            nc.vector.tensor_add(t3v, t2v[:, :, 0:2], t2v[:, :, 2:4])
            if blk == 0:
                nc.vector.tensor_add(s0[:], t3v[:, :, 0], t3v[:, :, 1])
            else:
                t4 = fold_scratch(JV)
                nc.vector.tensor_add(t4[:], t3v[:, :, 0], t3v[:, :, 1])
                nc.vector.tensor_add(s0[:], s0[:], t4[:])

        # ---- routing ----------------------------------------------
        w16 = rpool.tile([B, JV], f32, tag="w16")      # cumulative v
        bj = rpool.tile([B, JL, I], f16, tag="bj")     # b_r
        cE = rpool.tile([B, JL, I], f16, tag="cE")     # exp(b_r)
        Ssum = rpool.tile([B, JL], f32, tag="Ssum")
        Srec = rpool.tile([B, JL], f32, tag="Srec")
        sfac = rpool.tile([B, JL], f32, tag="sfac")
        s_acc = rpool.tile([B, JV], f32, tag="s_acc")
        sq = rpool.tile([B, JV], f32, tag="sq")
        n2 = rpool.tile([B, JL], f32, tag="n2")
        d1 = rpool.tile([B, JL], f32, tag="d1")
        r1 = rpool.tile([B, JL], f32, tag="r1")
        rt = rpool.tile([B, JL], f32, tag="rt")
        r2 = rpool.tile([B, JL], f32, tag="r2")
        fac = rpool.tile([B, JL], f32, tag="fac")
        vout = rpool.tile([B, JV], f32, tag="vout")

        def squash_j(j, s_ap, v_ap, zfac):
            """Per-j: v = s*zfac * g(n2), n2 = |s*zfac|^2, fused via
            n2 = zfac^2 * sum(s^2). All operands [B,1] or [B,V]."""
            jsl = slice(j, j + 1)
            nc.vector.tensor_mul(sq[:, j * V:(j + 1) * V], s_ap, s_ap)
            nc.vector.reduce_sum(
                n2[:, jsl],
                sq[:, j * V:(j + 1) * V].unsqueeze(1),
                axis=mybir.AxisListType.X)
            if zfac is not None:
                nc.vector.tensor_mul(r2[:, jsl], zfac, zfac)
                nc.vector.tensor_mul(n2[:, jsl], n2[:, jsl], r2[:, jsl])
            nc.scalar.add(d1[:, jsl], n2[:, jsl], 1.0)
            nc.vector.reciprocal(r1[:, jsl], d1[:, jsl])
            nc.scalar.activation(rt[:, jsl], n2[:, jsl],
                                 mybir.ActivationFunctionType.Sqrt,
                                 bias=eps_t[:])
            nc.vector.reciprocal(r2[:, jsl], rt[:, jsl])
            nc.vector.tensor_mul(fac[:, jsl], n2[:, jsl], r1[:, jsl])
            nc.vector.tensor_mul(fac[:, jsl], fac[:, jsl], r2[:, jsl])
            if zfac is not None:
                nc.vector.tensor_mul(fac[:, jsl], fac[:, jsl], zfac)
            fb = fac[:, jsl].broadcast_to([B, V])
            nc.vector.tensor_tensor(v_ap, s_ap, fb, op=MULT)

        COPY = mybir.ActivationFunctionType.Copy

        # v0 = squash(s0); w = v0  (per j so iter-1 ACT starts early)
        for j in range(JL):
            squash_j(j, s0[:, j * V:(j + 1) * V],
                     vout[:, j * V:(j + 1) * V], None)
            nc.vector.tensor_copy(w16[:, j * V:(j + 1) * V],
                                  vout[:, j * V:(j + 1) * V])

        # Routing: software-pipelined. Per (j,q) slot: 8 ACT scale-mul
        # products + DVE v-fold, then one deferred s-chunk keeps the DVE
        # busy while ACT works ahead. s-chunks of (r,j) run during the
        # (j+1) t-slots; per-j squash frees w16 so the next iteration's
        # ACT products never wait on a whole-iteration barrier.
        pending = []            # deferred closures: one s-chunk each

        def s_chunk(r, j, q):
            ceb = (cE[:, j, :].unsqueeze(1).broadcast_to([B, VQ, I]))
            pr = fold_scratch(VQ * I)
            prv = pr[:].rearrange("p (v i) -> p v i", i=I)
            nc.vector.tensor_tensor(
                prv, U[:, j, q * VQ:(q + 1) * VQ, :], ceb, op=MULT)
            cur = prv
            n = I
            while n > 2:
                nh = n // 2
                g = fold_scratch(VQ * nh)
                gv = g[:].rearrange("p (v i) -> p v i", i=nh)
                nc.vector.tensor_add(gv, cur[:, :, 0:nh], cur[:, :, nh:n])
                cur = gv
                n = nh
            nc.vector.tensor_add(
                s_acc[:].rearrange("p (j v) -> p j v", v=V)
                [:, j, q * VQ:(q + 1) * VQ],
                cur[:, :, 0], cur[:, :, 1])
            if q == V // VQ - 1:
                jsl = slice(j, j + 1)
                ssl = slice(j * V, (j + 1) * V)
                nc.vector.reciprocal(Srec[:, jsl], Ssum[:, jsl])
                nc.scalar.mul(sfac[:, jsl], Srec[:, jsl], float(I))
                squash_j(j, s_acc[:, ssl], vout[:, ssl], sfac[:, jsl])
                if r == 1:
                    nc.vector.tensor_add(w16[:, ssl], w16[:, ssl],
                                         vout[:, ssl])
                else:
                    nc.sync.dma_start(v2d.ap()[:, ssl], vout[:, ssl])

        def drain_one():
            if pending:
                pending.pop(0)()

        RP = 4                  # i-repeat chunk for the broadcast w tile

        for r in (1, 2):
            for j in range(JL):
                dve_self = (r == 1 and j == 0)
                for q in range(V // VQ):
                    drain_one()
                    tp = tpool.tile([B, VQ, I], f16, tag="tpq")
                    if dve_self:
                        # iter-1 j0: t-product on DVE (2x, 4-dim AP with
                        # stride-0 i-repeat) so the pipe self-starts
                        # instead of waiting ~25us for ACT
                        wE = fold_scratch(VQ * (I // RP))
                        wEv = wE[:].rearrange("p (v i) -> p v i",
                                              i=I // RP)
                        nc.vector.tensor_copy(
                            wEv, w16[:, j * V + q * VQ:j * V + (q + 1) * VQ]
                            .unsqueeze(2).broadcast_to([B, VQ, I // RP]))
                        nc.vector.tensor_tensor(
                            tp[:].rearrange("p v (k i) -> p v k i", k=RP),
                            U[:, j, q * VQ:(q + 1) * VQ, :]
                            .rearrange("p v (k i) -> p v k i", k=RP),
                            wEv.unsqueeze(2)
                            .broadcast_to([B, VQ, RP, I // RP]),
                            op=MULT)
                    else:
                        for m in range(VQ):
                            v = q * VQ + m
                            nc.scalar.activation(
                                tp[:, m, :], U[:, j, v, :], COPY,
                                scale=w16[:, j * V + v:j * V + v + 1])
                    # fold 8 -> 1 over v (outer halves, 2x)
                    h1 = fold_scratch(4 * I)
                    h1v = h1[:].rearrange("p (v i) -> p v i", i=I)
                    nc.vector.tensor_add(h1v, tp[:, 0:4, :], tp[:, 4:8, :])
                    h2 = fold_scratch(2 * I)
                    h2v = h2[:].rearrange("p (v i) -> p v i", i=I)
                    nc.vector.tensor_add(h2v, h1v[:, 0:2, :], h1v[:, 2:4, :])
                    if q == 0:
                        nc.vector.tensor_add(
                            bj[:, j, :], h2v[:, 0, :], h2v[:, 1, :])
                    else:
                        h3 = fold_scratch(I)
                        nc.vector.tensor_add(
                            h3[:], h2v[:, 0, :], h2v[:, 1, :])
                        nc.vector.tensor_add(
                            bj[:, j, :], bj[:, j, :], h3[:])
                # softmax denominator (|b| small: no max subtraction)
                nc.scalar.activation(cE[:, j, :], bj[:, j, :],
                                     mybir.ActivationFunctionType.Exp,
                                     accum_out=Ssum[:, j:j + 1])
                for q in range(V // VQ):
                    pending.append(
                        lambda r=r, j=j, q=q: s_chunk(r, j, q))
        while pending:
            drain_one()
        stack.close()

    nc.compile()
    return nc


def _get_program():
    if "nc" not in _cache:
        _cache["nc"] = _build_program()
    return _cache["nc"]


def _prep_inputs(x, W):
    """Host-side shard + transpose + fp16 cast."""
    u = np.ascontiguousarray(x[..., 0])                   # [B, I, D] f32
    xt = np.ascontiguousarray(u.transpose(2, 1, 0)).astype(np.float16)
    xa_np = np.ascontiguousarray(xt[:DP])                 # [128, I, B]
    xb_np = np.ascontiguousarray(xt[DP:])
    W0 = W[0]                                             # [I, J, D, V]
    in_maps = []
    for c in range(NCORES):
        Wc = W0[:, c * JL:(c + 1) * JL]                   # [I, JL, D, V]
        Wt = Wc.transpose(2, 0, 1, 3)                     # [D, I, JL, V]
        Wt = Wt.reshape(D, I, JV).astype(np.float16)
        in_maps.append({
            "xa": xa_np,
            "xb": xb_np,
            "wa": np.ascontiguousarray(Wt[:DP]),
            "wb": np.ascontiguousarray(Wt[DP:]),
        })
    return in_maps


def run_cores(x, W, trace=False):
    from concourse import bass_utils
    nc = _get_program()
    in_maps = _prep_inputs(x, W)
    res = bass_utils.run_bass_kernel_spmd(
        nc, in_maps, core_ids=list(range(NCORES)), trace=trace)
    return res


def kernel(x, W):
    x = np.asarray(x)
    W = np.asarray(W)
    res = run_cores(x, W, trace=False)
    out = np.empty((B, J, V, 1), dtype=np.float32)
    for c in range(NCORES):
        vc = res.results[c]["v2"].reshape(B, JL, V)
        out[:, c * JL:(c + 1) * JL, :, 0] = vc
    return out


# revision 14
# speedup vs baseline: 1.0085x; 1.0085x over previous
"""CapsNet dynamic-routing layer on 8 Trainium2 NeuronCores.

Sharding: tensor-parallel over num_caps_j (J=32 -> 4 per core). Every
(batch, j) pair's routing is independent, so there are no collectives:
each core computes u_hat[:, :, j_shard, :] plus 3 routing iterations and
returns v_J[:, j_shard, :].

Phase 1 (einsum, DMA-bound ~67MB/core): 1024 PE matmuls
(stationary = x chunk [d,b], moving = W [d,jv], N=128) accumulate
u_hat into PSUM; the scalar engine copies each PSUM block to SBUF in
TRANSPOSED layout U[b, j, v, i] (i innermost). s0 = sum_i u_hat is
folded per block on the otherwise-idle DVE.

Phase 2 (routing, 2 iterations). The [b, j, v, i] layout gives:
  - t-pass product w_v*U_v on the SCALAR engine (scale is the
    per-partition [B,1] AP it natively supports), v-fold on DVE at
    2x (v is an outer axis -> contiguous halves).
  - exp(b_r) is only [B, JL, I] (2M elems vs 8.4M broadcast before).
  - s-pass product U * exp(b) on DVE at 2x: exp(b)[B,I] broadcast
    over v is a stride-0 OUTER dim, which keeps the innermost stride
    at 1 (the 2x_1p requirement); i-fold halves stay contiguous.
"""

import sys

if "/opt/trn_rl_repo" not in sys.path:
    sys.path.insert(0, "/opt/trn_rl_repo")

import numpy as np

B, I, D, J, V = 128, 512, 256, 32, 32
NCORES = 8
JL = J // NCORES          # 4 j's per core
JV = JL * V               # 128
DP = 128                  # contraction chunk (partitions)
EPS = 1e-9
IBLK = 16                 # i-block per DMA tile / PSUM block
VQ = 8                    # v-quarter for t/s pass tiles

_cache = {}


def _build_program():
    import concourse.tile as tile
    from concourse import bacc, mybir

    f16 = mybir.dt.float16
    f32 = mybir.dt.float32
    MULT = mybir.AluOpType.mult

    nc = bacc.Bacc("TRN2", target_bir_lowering=False, debug=False,
                   num_devices=NCORES)

    xa = nc.dram_tensor("xa", [DP, I, B], f16, kind="ExternalInput")
    xb = nc.dram_tensor("xb", [DP, I, B], f16, kind="ExternalInput")
    wa = nc.dram_tensor("wa", [DP, I, JV], f16, kind="ExternalInput")
    wb = nc.dram_tensor("wb", [DP, I, JV], f16, kind="ExternalInput")
    v2d = nc.dram_tensor("v2", [B, JV], f32, kind="ExternalOutput")

    with tile.TileContext(nc) as tc:
        from contextlib import ExitStack
        stack = ExitStack()
        upool = stack.enter_context(tc.tile_pool(name="uhat", bufs=1))
        xwpool = stack.enter_context(tc.tile_pool(name="xw", bufs=2))
        pspool = stack.enter_context(
            tc.tile_pool(name="psum", bufs=2, space="PSUM"))
        rpool = stack.enter_context(tc.tile_pool(name="rout", bufs=1))
        tpool = stack.enter_context(tc.tile_pool(name="tp", bufs=2))
        fpool = stack.enter_context(tc.tile_pool(name="fold", bufs=1))

        eps_t = rpool.tile([B, 1], f32, tag="eps")
        nc.gpsimd.memset(eps_t[:], EPS)

        # u_hat, [b, j, v, i] fp16 (i innermost)
        U = upool.tile([B, JL, V, I], f16)
        s0 = rpool.tile([B, JV], f32, tag="s0")

        def fold_scratch(n):
            """Flat [B, n] fp16 scratch keyed by size."""
            return fpool.tile([B, n], f16, tag=f"f{n}", name=f"fs{n}")

        # ---- einsum with transposed PSUM->SBUF copy + inline s0 ----
        for blk in range(I // IBLK):
            i0 = blk * IBLK
            xa_t = xwpool.tile([DP, IBLK, B], f16, tag="xa")
            nc.sync.dma_start(xa_t[:], xa.ap()[:, i0:i0 + IBLK, :])
            xb_t = xwpool.tile([DP, IBLK, B], f16, tag="xb")
            nc.sync.dma_start(xb_t[:], xb.ap()[:, i0:i0 + IBLK, :])
            wa_t = xwpool.tile([DP, IBLK, JV], f16, tag="wa")
            nc.gpsimd.dma_start(wa_t[:], wa.ap()[:, i0:i0 + IBLK, :])
            wb_t = xwpool.tile([DP, IBLK, JV], f16, tag="wb")
            nc.gpsimd.dma_start(wb_t[:], wb.ap()[:, i0:i0 + IBLK, :])

            ps = pspool.tile([B, IBLK, JV], f32)   # four 2KB banks
            for k in range(IBLK):
                nc.tensor.matmul(
                    ps[:, k, :], xa_t[:, k, :], wa_t[:, k, :],
                    start=True, stop=False)
                nc.tensor.matmul(
                    ps[:, k, :], xb_t[:, k, :], wb_t[:, k, :],
                    start=False, stop=True)
            # transpose (i, j, v) -> (j, v, i) on the scalar engine
            nc.scalar.copy(
                U[:, :, :, i0:i0 + IBLK],
                ps.rearrange("p i (j v) -> p j v i", j=JL))

            # s0 partial: fold this i-block (innermost halves, 2x)
            ub = U[:, :, :, i0:i0 + IBLK]
            t1 = fold_scratch(JV * 8)
            t1v = t1[:].rearrange("p (jv i) -> p jv i", i=8)
            nc.vector.tensor_add(t1v, ub.rearrange("p j v i -> p (j v) i")
                                 [:, :, 0:8],
                                 ub.rearrange("p j v i -> p (j v) i")
                                 [:, :, 8:16])
            t2 = fold_scratch(JV * 4)
            t2v = t2[:].rearrange("p (jv i) -> p jv i", i=4)
            nc.vector.tensor_add(t2v, t1v[:, :, 0:4], t1v[:, :, 4:8])
            t3 = fold_scratch(JV * 2)
            t3v = t3[:].rearrange("p (jv i) -> p jv i", i=2)
            nc.vector.tensor_add(t3v, t2v[:, :, 0:2], t2v[:, :, 2:4])
            if blk == 0:
                nc.vector.tensor_add(s0[:], t3v[:, :, 0], t3v[:, :, 1])
            else:
                t4 = fold_scratch(JV)
                nc.vector.tensor_add(t4[:], t3v[:, :, 0], t3v[:, :, 1])
                nc.vector.tensor_add(s0[:], s0[:], t4[:])

        # ---- routing ----------------------------------------------
        w16 = rpool.tile([B, JV], f32, tag="w16")      # cumulative v
        bj = rpool.tile([B, JL, I], f16, tag="bj")     # b_r
        cE = rpool.tile([B, JL, I], f16, tag="cE")     # exp(b_r)
        Ssum = rpool.tile([B, JL], f32, tag="Ssum")
        Srec = rpool.tile([B, JL], f32, tag="Srec")
        sfac = rpool.tile([B, JL], f32, tag="sfac")
        s_acc = rpool.tile([B, JV], f32, tag="s_acc")
        sq = rpool.tile([B, JV], f32, tag="sq")
        n2 = rpool.tile([B, JL], f32, tag="n2")
        d1 = rpool.tile([B, JL], f32, tag="d1")
        r1 = rpool.tile([B, JL], f32, tag="r1")
        rt = rpool.tile([B, JL], f32, tag="rt")
        r2 = rpool.tile([B, JL], f32, tag="r2")
        fac = rpool.tile([B, JL], f32, tag="fac")
        vout = rpool.tile([B, JV], f32, tag="vout")

        def squash_j(j, s_ap, v_ap, zfac):
            """Per-j: v = s*zfac * g(n2), n2 = |s*zfac|^2, fused via
            n2 = zfac^2 * sum(s^2). All operands [B,1] or [B,V]."""
            jsl = slice(j, j + 1)
            nc.vector.tensor_mul(sq[:, j * V:(j + 1) * V], s_ap, s_ap)
            nc.vector.reduce_sum(
                n2[:, jsl],
                sq[:, j * V:(j + 1) * V].unsqueeze(1),
                axis=mybir.AxisListType.X)
            if zfac is not None:
                nc.vector.tensor_mul(r2[:, jsl], zfac, zfac)
                nc.vector.tensor_mul(n2[:, jsl], n2[:, jsl], r2[:, jsl])
            nc.vector.tensor_scalar_add(d1[:, jsl], n2[:, jsl], 1.0)
            nc.vector.reciprocal(r1[:, jsl], d1[:, jsl])
            # rsqrt(n2+eps) = exp(-0.5*ln(n2+eps)): keeps the scalar
            # engine inside one act-table set (ln/exp/copy) -- a Sqrt
            # here forces a ~1.3us table reload around every exp
            nc.scalar.activation(rt[:, jsl], n2[:, jsl],
                                 mybir.ActivationFunctionType.Ln,
                                 bias=eps_t[:])
            nc.scalar.activation(r2[:, jsl], rt[:, jsl],
                                 mybir.ActivationFunctionType.Exp,
                                 scale=-0.5)
            nc.vector.tensor_mul(fac[:, jsl], n2[:, jsl], r1[:, jsl])
            nc.vector.tensor_mul(fac[:, jsl], fac[:, jsl], r2[:, jsl])
            if zfac is not None:
                nc.vector.tensor_mul(fac[:, jsl], fac[:, jsl], zfac)
            fb = fac[:, jsl].broadcast_to([B, V])
            nc.vector.tensor_tensor(v_ap, s_ap, fb, op=MULT)

        COPY = mybir.ActivationFunctionType.Copy

        # v0 = squash(s0); w = v0  (per j so iter-1 ACT starts early)
        for j in range(JL):
            squash_j(j, s0[:, j * V:(j + 1) * V],
                     vout[:, j * V:(j + 1) * V], None)
            nc.vector.tensor_copy(w16[:, j * V:(j + 1) * V],
                                  vout[:, j * V:(j + 1) * V])

        # Routing: software-pipelined. Per (j,q) slot: 8 ACT scale-mul
        # products + DVE v-fold, then one deferred s-chunk keeps the DVE
        # busy while ACT works ahead. s-chunks of (r,j) run during the
        # (j+1) t-slots; per-j squash frees w16 so the next iteration's
        # ACT products never wait on a whole-iteration barrier.
        pending = []            # deferred closures: one s-chunk each

        def s_chunk(r, j, q):
            ceb = (cE[:, j, :].unsqueeze(1).broadcast_to([B, VQ, I]))
            pr = fold_scratch(VQ * I)
            prv = pr[:].rearrange("p (v i) -> p v i", i=I)
            nc.vector.tensor_tensor(
                prv, U[:, j, q * VQ:(q + 1) * VQ, :], ceb, op=MULT)
            cur = prv
            n = I
            while n > 2:
                nh = n // 2
                g = fold_scratch(VQ * nh)
                gv = g[:].rearrange("p (v i) -> p v i", i=nh)
                nc.vector.tensor_add(gv, cur[:, :, 0:nh], cur[:, :, nh:n])
                cur = gv
                n = nh
            nc.vector.tensor_add(
                s_acc[:].rearrange("p (j v) -> p j v", v=V)
                [:, j, q * VQ:(q + 1) * VQ],
                cur[:, :, 0], cur[:, :, 1])
            if q == V // VQ - 1:
                jsl = slice(j, j + 1)
                ssl = slice(j * V, (j + 1) * V)
                nc.vector.reciprocal(Srec[:, jsl], Ssum[:, jsl])
                nc.scalar.mul(sfac[:, jsl], Srec[:, jsl], float(I))
                squash_j(j, s_acc[:, ssl], vout[:, ssl], sfac[:, jsl])
                if r == 1:
                    nc.vector.tensor_add(w16[:, ssl], w16[:, ssl],
                                         vout[:, ssl])
                else:
                    nc.sync.dma_start(v2d.ap()[:, ssl], vout[:, ssl])

        def drain_one():
            if pending:
                pending.pop(0)()

        RP = 4                  # i-repeat chunk for the broadcast w tile

        for r in (1, 2):
            for j in range(JL):
                dve_self = (r == 1 and j == 0)
                for q in range(V // VQ):
                    tp = tpool.tile([B, VQ, I], f16, tag="tpq")
                    if dve_self:
                        # iter-1 j0: t-product on DVE (2x, 4-dim AP with
                        # stride-0 i-repeat) so the pipe self-starts
                        # instead of waiting ~25us for ACT
                        wE = fold_scratch(VQ * (I // RP))
                        wEv = wE[:].rearrange("p (v i) -> p v i",
                                              i=I // RP)
                        nc.vector.tensor_copy(
                            wEv, w16[:, j * V + q * VQ:j * V + (q + 1) * VQ]
                            .unsqueeze(2).broadcast_to([B, VQ, I // RP]))
                        nc.vector.tensor_tensor(
                            tp[:].rearrange("p v (k i) -> p v k i", k=RP),
                            U[:, j, q * VQ:(q + 1) * VQ, :]
                            .rearrange("p v (k i) -> p v k i", k=RP),
                            wEv.unsqueeze(2)
                            .broadcast_to([B, VQ, RP, I // RP]),
                            op=MULT)
                    else:
                        for m in range(VQ):
                            v = q * VQ + m
                            nc.scalar.activation(
                                tp[:, m, :], U[:, j, v, :], COPY,
                                scale=w16[:, j * V + v:j * V + v + 1])
                    # fold 8 -> 1 over v (outer halves, 2x)
                    h1 = fold_scratch(4 * I)
                    h1v = h1[:].rearrange("p (v i) -> p v i", i=I)
                    nc.vector.tensor_add(h1v, tp[:, 0:4, :], tp[:, 4:8, :])
                    h2 = fold_scratch(2 * I)
                    h2v = h2[:].rearrange("p (v i) -> p v i", i=I)
                    nc.vector.tensor_add(h2v, h1v[:, 0:2, :], h1v[:, 2:4, :])
                    if q == 0:
                        nc.vector.tensor_add(
                            bj[:, j, :], h2v[:, 0, :], h2v[:, 1, :])
                    else:
                        h3 = fold_scratch(I)
                        nc.vector.tensor_add(
                            h3[:], h2v[:, 0, :], h2v[:, 1, :])
                        nc.vector.tensor_add(
                            bj[:, j, :], bj[:, j, :], h3[:])
                    drain_one()
                # softmax denominator (|b| small: no max subtraction)
                nc.scalar.activation(cE[:, j, :], bj[:, j, :],
                                     mybir.ActivationFunctionType.Exp,
                                     accum_out=Ssum[:, j:j + 1])
                for q in range(V // VQ):
                    pending.append(
                        lambda r=r, j=j, q=q: s_chunk(r, j, q))
        while pending:
            drain_one()
        stack.close()

    nc.compile()
    return nc


def _get_program():
    if "nc" not in _cache:
        _cache["nc"] = _build_program()
    return _cache["nc"]


def _prep_inputs(x, W):
    """Host-side shard + transpose + fp16 cast."""
    u = np.ascontiguousarray(x[..., 0])                   # [B, I, D] f32
    xt = np.ascontiguousarray(u.transpose(2, 1, 0)).astype(np.float16)
    xa_np = np.ascontiguousarray(xt[:DP])                 # [128, I, B]
    xb_np = np.ascontiguousarray(xt[DP:])
    W0 = W[0]                                             # [I, J, D, V]
    in_maps = []
    for c in range(NCORES):
        Wc = W0[:, c * JL:(c + 1) * JL]                   # [I, JL, D, V]
        Wt = Wc.transpose(2, 0, 1, 3)                     # [D, I, JL, V]
        Wt = Wt.reshape(D, I, JV).astype(np.float16)
        in_maps.append({
            "xa": xa_np,
            "xb": xb_np,
            "wa": np.ascontiguousarray(Wt[:DP]),
            "wb": np.ascontiguousarray(Wt[DP:]),
        })
    return in_maps


def run_cores(x, W, trace=False):
    from concourse import bass_utils
    nc = _get_program()
    in_maps = _prep_inputs(x, W)
    res = bass_utils.run_bass_kernel_spmd(
        nc, in_maps, core_ids=list(range(NCORES)), trace=trace)
    return res


def kernel(x, W):
    x = np.asarray(x)
    W = np.asarray(W)
    res = run_cores(x, W, trace=False)
    out = np.empty((B, J, V, 1), dtype=np.float32)
    for c in range(NCORES):
        vc = res.results[c]["v2"].reshape(B, JL, V)
        out[:, c * JL:(c + 1) * JL, :, 0] = vc
    return out


# revision 16
# speedup vs baseline: 1.0621x; 1.0531x over previous
"""CapsNet dynamic-routing layer on 8 Trainium2 NeuronCores.

Sharding: tensor-parallel over num_caps_j (J=32 -> 4 per core). Every
(batch, j) pair's routing is independent, so there are no collectives:
each core computes u_hat[:, :, j_shard, :] plus 3 routing iterations and
returns v_J[:, j_shard, :].

Phase 1 (einsum, DMA-bound ~67MB/core): 1024 PE matmuls
(stationary = x chunk [d,b], moving = W [d,jv], N=128) accumulate
u_hat into PSUM; the scalar engine copies each PSUM block to SBUF in
TRANSPOSED layout U[b, j, v, i] (i innermost). s0 = sum_i u_hat is
folded per block on the otherwise-idle DVE.

Phase 2 (routing, 2 iterations). The [b, j, v, i] layout gives:
  - t-pass product w_v*U_v on the SCALAR engine (scale is the
    per-partition [B,1] AP it natively supports), v-fold on DVE at
    2x (v is an outer axis -> contiguous halves).
  - exp(b_r) is only [B, JL, I] (2M elems vs 8.4M broadcast before).
  - s-pass product U * exp(b) on DVE at 2x: exp(b)[B,I] broadcast
    over v is a stride-0 OUTER dim, which keeps the innermost stride
    at 1 (the 2x_1p requirement); i-fold halves stay contiguous.
"""

import sys

if "/opt/trn_rl_repo" not in sys.path:
    sys.path.insert(0, "/opt/trn_rl_repo")

import numpy as np

B, I, D, J, V = 128, 512, 256, 32, 32
NCORES = 8
JL = J // NCORES          # 4 j's per core
JV = JL * V               # 128
DP = 128                  # contraction chunk (partitions)
EPS = 1e-9
IBLK = 16                 # i-block per DMA tile / PSUM block
VQ = 8                    # v-quarter for t/s pass tiles

_cache = {}


def _build_program():
    import concourse.tile as tile
    from concourse import bacc, mybir

    f16 = mybir.dt.float16
    f32 = mybir.dt.float32
    MULT = mybir.AluOpType.mult

    nc = bacc.Bacc("TRN2", target_bir_lowering=False, debug=False,
                   num_devices=NCORES)

    xa = nc.dram_tensor("xa", [DP, I, B], f16, kind="ExternalInput")
    xb = nc.dram_tensor("xb", [DP, I, B], f16, kind="ExternalInput")
    wa = nc.dram_tensor("wa", [DP, I, JV], f16, kind="ExternalInput")
    wb = nc.dram_tensor("wb", [DP, I, JV], f16, kind="ExternalInput")
    v2d = nc.dram_tensor("v2", [B, JV], f32, kind="ExternalOutput")

    with tile.TileContext(nc) as tc:
        from contextlib import ExitStack
        stack = ExitStack()
        upool = stack.enter_context(tc.tile_pool(name="uhat", bufs=1))
        xwpool = stack.enter_context(tc.tile_pool(name="xw", bufs=2))
        pspool = stack.enter_context(
            tc.tile_pool(name="psum", bufs=2, space="PSUM"))
        rpool = stack.enter_context(tc.tile_pool(name="rout", bufs=1))
        tpool = stack.enter_context(tc.tile_pool(name="tp", bufs=2))
        fpool = stack.enter_context(tc.tile_pool(name="fold", bufs=1))

        eps_t = rpool.tile([B, 1], f32, tag="eps")
        nc.gpsimd.memset(eps_t[:], EPS)

        # u_hat, [b, j, v, i] fp16 (i innermost)
        U = upool.tile([B, JL, V, I], f16)
        s0 = rpool.tile([B, JV], f32, tag="s0")

        def fold_scratch(n):
            """Flat [B, n] fp16 scratch keyed by size."""
            return fpool.tile([B, n], f16, tag=f"f{n}", name=f"fs{n}")

        # ---- einsum with transposed PSUM->SBUF copy + inline s0 ----
        for blk in range(I // IBLK):
            i0 = blk * IBLK
            xa_t = xwpool.tile([DP, IBLK, B], f16, tag="xa")
            nc.sync.dma_start(xa_t[:], xa.ap()[:, i0:i0 + IBLK, :])
            xb_t = xwpool.tile([DP, IBLK, B], f16, tag="xb")
            nc.sync.dma_start(xb_t[:], xb.ap()[:, i0:i0 + IBLK, :])
            wa_t = xwpool.tile([DP, IBLK, JV], f16, tag="wa")
            nc.gpsimd.dma_start(wa_t[:], wa.ap()[:, i0:i0 + IBLK, :])
            wb_t = xwpool.tile([DP, IBLK, JV], f16, tag="wb")
            nc.gpsimd.dma_start(wb_t[:], wb.ap()[:, i0:i0 + IBLK, :])

            ps = pspool.tile([B, IBLK, JV], f32)   # four 2KB banks
            for k in range(IBLK):
                nc.tensor.matmul(
                    ps[:, k, :], xa_t[:, k, :], wa_t[:, k, :],
                    start=True, stop=False)
                nc.tensor.matmul(
                    ps[:, k, :], xb_t[:, k, :], wb_t[:, k, :],
                    start=False, stop=True)
            # transpose (i, j, v) -> (j, v, i) on the scalar engine
            nc.scalar.copy(
                U[:, :, :, i0:i0 + IBLK],
                ps.rearrange("p i (j v) -> p j v i", j=JL))

            # s0 partial: fold this i-block (innermost halves, 2x)
            ub = U[:, :, :, i0:i0 + IBLK]
            t1 = fold_scratch(JV * 8)
            t1v = t1[:].rearrange("p (jv i) -> p jv i", i=8)
            nc.vector.tensor_add(t1v, ub.rearrange("p j v i -> p (j v) i")
                                 [:, :, 0:8],
                                 ub.rearrange("p j v i -> p (j v) i")
                                 [:, :, 8:16])
            t2 = fold_scratch(JV * 4)
            t2v = t2[:].rearrange("p (jv i) -> p jv i", i=4)
            nc.vector.tensor_add(t2v, t1v[:, :, 0:4], t1v[:, :, 4:8])
            t3 = fold_scratch(JV * 2)
            t3v = t3[:].rearrange("p (jv i) -> p jv i", i=2)
            nc.vector.tensor_add(t3v, t2v[:, :, 0:2], t2v[:, :, 2:4])
            if blk == 0:
                nc.vector.tensor_add(s0[:], t3v[:, :, 0], t3v[:, :, 1])
            else:
                t4 = fold_scratch(JV)
                nc.vector.tensor_add(t4[:], t3v[:, :, 0], t3v[:, :, 1])
                nc.vector.tensor_add(s0[:], s0[:], t4[:])

        # ---- routing ----------------------------------------------
        w16 = rpool.tile([B, JV], f32, tag="w16")      # cumulative v
        bj = rpool.tile([B, JL, I], f16, tag="bj")     # b_r
        cE = rpool.tile([B, JL, I], f16, tag="cE")     # exp(b_r)
        Ssum = rpool.tile([B, JL], f32, tag="Ssum")
        Srec = rpool.tile([B, JL], f32, tag="Srec")
        sfac = rpool.tile([B, JL], f32, tag="sfac")
        s_acc = rpool.tile([B, JV], f32, tag="s_acc")
        sq = rpool.tile([B, JV], f32, tag="sq")
        n2 = rpool.tile([B, JL], f32, tag="n2")
        d1 = rpool.tile([B, JL], f32, tag="d1")
        r1 = rpool.tile([B, JL], f32, tag="r1")
        rt = rpool.tile([B, JL], f32, tag="rt")
        r2 = rpool.tile([B, JL], f32, tag="r2")
        fac = rpool.tile([B, JL], f32, tag="fac")
        vout = rpool.tile([B, JV], f32, tag="vout")

        def squash_j(j, s_ap, v_ap, zfac):
            """Per-j: v = s*zfac * g(n2), n2 = |s*zfac|^2, fused via
            n2 = zfac^2 * sum(s^2). All operands [B,1] or [B,V]."""
            jsl = slice(j, j + 1)
            nc.vector.tensor_mul(sq[:, j * V:(j + 1) * V], s_ap, s_ap)
            nc.vector.reduce_sum(
                n2[:, jsl],
                sq[:, j * V:(j + 1) * V].unsqueeze(1),
                axis=mybir.AxisListType.X)
            if zfac is not None:
                nc.vector.tensor_mul(r2[:, jsl], zfac, zfac)
                nc.vector.tensor_mul(n2[:, jsl], n2[:, jsl], r2[:, jsl])
            nc.vector.tensor_scalar_add(d1[:, jsl], n2[:, jsl], 1.0)
            nc.vector.reciprocal(r1[:, jsl], d1[:, jsl])
            nc.scalar.activation(rt[:, jsl], n2[:, jsl],
                                 mybir.ActivationFunctionType.Sqrt,
                                 bias=eps_t[:])
            nc.vector.reciprocal(r2[:, jsl], rt[:, jsl])
            nc.vector.tensor_mul(fac[:, jsl], n2[:, jsl], r1[:, jsl])
            nc.vector.tensor_mul(fac[:, jsl], fac[:, jsl], r2[:, jsl])
            if zfac is not None:
                nc.vector.tensor_mul(fac[:, jsl], fac[:, jsl], zfac)
            fb = fac[:, jsl].broadcast_to([B, V])
            nc.vector.tensor_tensor(v_ap, s_ap, fb, op=MULT)

        COPY = mybir.ActivationFunctionType.Copy

        # v0 = squash(s0); w = v0  (per j so iter-1 ACT starts early)
        for j in range(JL):
            squash_j(j, s0[:, j * V:(j + 1) * V],
                     vout[:, j * V:(j + 1) * V], None)
            nc.vector.tensor_copy(w16[:, j * V:(j + 1) * V],
                                  vout[:, j * V:(j + 1) * V])

        # Routing: software-pipelined. Per (j,q) slot: 8 ACT scale-mul
        # products + DVE v-fold, then one deferred s-chunk keeps the DVE
        # busy while ACT works ahead. s-chunks of (r,j) run during the
        # (j+1) t-slots; per-j squash frees w16 so the next iteration's
        # ACT products never wait on a whole-iteration barrier.
        pending = []            # deferred closures: one s-chunk each

        def s_chunk(r, j, q):
            ceb = (cE[:, j, :].unsqueeze(1).broadcast_to([B, VQ, I]))
            pr = fold_scratch(VQ * I)
            prv = pr[:].rearrange("p (v i) -> p v i", i=I)
            nc.vector.tensor_tensor(
                prv, U[:, j, q * VQ:(q + 1) * VQ, :], ceb, op=MULT)
            cur = prv
            n = I
            while n > 2:
                nh = n // 2
                g = fold_scratch(VQ * nh)
                gv = g[:].rearrange("p (v i) -> p v i", i=nh)
                nc.vector.tensor_add(gv, cur[:, :, 0:nh], cur[:, :, nh:n])
                cur = gv
                n = nh
            nc.vector.tensor_add(
                s_acc[:].rearrange("p (j v) -> p j v", v=V)
                [:, j, q * VQ:(q + 1) * VQ],
                cur[:, :, 0], cur[:, :, 1])
            if q == V // VQ - 1 and j % 2 == 1:
                # squash j-pairs together so their Sqrt instructions sit
                # adjacent in the ACT FIFO: one act-table round trip per
                # pair instead of one per j
                for jj in (j - 1, j):
                    jsl = slice(jj, jj + 1)
                    ssl = slice(jj * V, (jj + 1) * V)
                    nc.vector.reciprocal(Srec[:, jsl], Ssum[:, jsl])
                    nc.scalar.mul(sfac[:, jsl], Srec[:, jsl], float(I))
                    squash_j(jj, s_acc[:, ssl], vout[:, ssl],
                             sfac[:, jsl])
                    if r == 1:
                        nc.vector.tensor_add(w16[:, ssl], w16[:, ssl],
                                             vout[:, ssl])
                    else:
                        nc.sync.dma_start(v2d.ap()[:, ssl], vout[:, ssl])

        def drain_one():
            if pending:
                pending.pop(0)()

        RP = 4                  # i-repeat chunk for the broadcast w tile

        for r in (1, 2):
            for j in range(JL):
                dve_self = (r == 1 and j == 0)
                for q in range(V // VQ):
                    tp = tpool.tile([B, VQ, I], f16, tag="tpq")
                    if dve_self:
                        # iter-1 j0: t-product on DVE (2x, 4-dim AP with
                        # stride-0 i-repeat) so the pipe self-starts
                        # instead of waiting ~25us for ACT
                        wE = fold_scratch(VQ * (I // RP))
                        wEv = wE[:].rearrange("p (v i) -> p v i",
                                              i=I // RP)
                        nc.vector.tensor_copy(
                            wEv, w16[:, j * V + q * VQ:j * V + (q + 1) * VQ]
                            .unsqueeze(2).broadcast_to([B, VQ, I // RP]))
                        nc.vector.tensor_tensor(
                            tp[:].rearrange("p v (k i) -> p v k i", k=RP),
                            U[:, j, q * VQ:(q + 1) * VQ, :]
                            .rearrange("p v (k i) -> p v k i", k=RP),
                            wEv.unsqueeze(2)
                            .broadcast_to([B, VQ, RP, I // RP]),
                            op=MULT)
                    else:
                        for m in range(VQ):
                            v = q * VQ + m
                            nc.scalar.activation(
                                tp[:, m, :], U[:, j, v, :], COPY,
                                scale=w16[:, j * V + v:j * V + v + 1])
                    # fold 8 -> 1 over v (outer halves, 2x)
                    h1 = fold_scratch(4 * I)
                    h1v = h1[:].rearrange("p (v i) -> p v i", i=I)
                    nc.vector.tensor_add(h1v, tp[:, 0:4, :], tp[:, 4:8, :])
                    h2 = fold_scratch(2 * I)
                    h2v = h2[:].rearrange("p (v i) -> p v i", i=I)
                    nc.vector.tensor_add(h2v, h1v[:, 0:2, :], h1v[:, 2:4, :])
                    if q == 0:
                        nc.vector.tensor_add(
                            bj[:, j, :], h2v[:, 0, :], h2v[:, 1, :])
                    else:
                        h3 = fold_scratch(I)
                        nc.vector.tensor_add(
                            h3[:], h2v[:, 0, :], h2v[:, 1, :])
                        nc.vector.tensor_add(
                            bj[:, j, :], bj[:, j, :], h3[:])
                    drain_one()
                # softmax denominator (|b| small: no max subtraction)
                nc.scalar.activation(cE[:, j, :], bj[:, j, :],
                                     mybir.ActivationFunctionType.Exp,
                                     accum_out=Ssum[:, j:j + 1])
                for q in range(V // VQ):
                    pending.append(
                        lambda r=r, j=j, q=q: s_chunk(r, j, q))
        while pending:
            drain_one()
        stack.close()

    nc.compile()
    return nc


def _get_program():
    if "nc" not in _cache:
        _cache["nc"] = _build_program()
    return _cache["nc"]


def _prep_inputs(x, W):
    """Host-side shard + transpose + fp16 cast."""
    u = np.ascontiguousarray(x[..., 0])                   # [B, I, D] f32
    xt = np.ascontiguousarray(u.transpose(2, 1, 0)).astype(np.float16)
    xa_np = np.ascontiguousarray(xt[:DP])                 # [128, I, B]
    xb_np = np.ascontiguousarray(xt[DP:])
    W0 = W[0]                                             # [I, J, D, V]
    in_maps = []
    for c in range(NCORES):
        Wc = W0[:, c * JL:(c + 1) * JL]                   # [I, JL, D, V]
        Wt = Wc.transpose(2, 0, 1, 3)                     # [D, I, JL, V]
        Wt = Wt.reshape(D, I, JV).astype(np.float16)
        in_maps.append({
            "xa": xa_np,
            "xb": xb_np,
            "wa": np.ascontiguousarray(Wt[:DP]),
            "wb": np.ascontiguousarray(Wt[DP:]),
        })
    return in_maps


def run_cores(x, W, trace=False):
    from concourse import bass_utils
    nc = _get_program()
    in_maps = _prep_inputs(x, W)
    res = bass_utils.run_bass_kernel_spmd(
        nc, in_maps, core_ids=list(range(NCORES)), trace=trace)
    return res


def kernel(x, W):
    x = np.asarray(x)
    W = np.asarray(W)
    res = run_cores(x, W, trace=False)
    out = np.empty((B, J, V, 1), dtype=np.float32)
    for c in range(NCORES):
        vc = res.results[c]["v2"].reshape(B, JL, V)
        out[:, c * JL:(c + 1) * JL, :, 0] = vc
    return out
